# revision 1
# baseline (speedup 1.0000x reference)
"""Trainium2 Bass kernel for nn_Decoding_33019708572164 (ragged spline decoder ELBO).

Strategy (8 NeuronCores, data-parallel over the 1M ragged cuts):
  - Each core owns 125 cells (= 62500 rows of the height_delta table).
  - Cuts are routed to cores by their height-row index r = cut_local_cellxgene_ix
    (core = r // 62500); within a core, cuts are bucketed by (table-half, spline
    bin b) so the per-cut 2-point interpolation becomes static column slices and
    int16 gather indices stay in range.
  - Phase A (device): PE builds the per-core log-height table
    A[r_loc, k] = latent[c] . hsw[genes_oi[g], :, k]  (bf16, DRAM, rows padded
    to 256 elems for dma_gather's 256B-multiple element size).
  - Phase B: dma_gather row gathers (A row by r_loc, spline row by gene idx j),
    wide add + exp + trapezoid reduce on the [:129] slice, interpolation from
    two static columns per bucket, masked sum.
  - Phase C: the softmax/overall term is rewritten as sum(counts * log_softmax)
    with counts = histogram of cut_localcellxgene_ix (host bincount); each core
    computes its 125-cell slab of the [1000, 5000] log-softmax on PE/ACT/DVE.
  - Host: sums the 8 per-core partial pairs and adds the exact constant
    N * (log 128 + log 5000).
"""

import sys

if "/opt/trn_rl_repo" not in sys.path:
    sys.path.insert(0, "/opt/trn_rl_repo")

import numpy as np
import ml_dtypes

N_CORES = 8
N_CELLS = 1000
N_GOI = 500
N_GT = 5000
NL = 10
K = 128
NK = 129
ES = 256                          # padded row length (bf16) = 512B
CPC = N_CELLS // N_CORES          # cells per core = 125
RPC = CPC * N_GOI                 # table rows per core = 62500
HALF = RPC // 2                   # 31250 rows per half-table (int16 idx range)
SLOT = 128                        # cuts per slot (partition dim)
GS = 64                           # slots per gather group (8192 cuts)
GC = GS * SLOT                    # cuts per group
BF16 = ml_dtypes.bfloat16

_PROGRAM_CACHE = {}


def _host_prep(latent, cut_coordinates, genes_oi, cut_local_cellxgene_ix,
               cut_localcellxgene_ix, cut_local_gene_ix, height_slope_w,
               overall_slope_w, overall_baseline, spline_baseline):
    latent = np.asarray(latent, np.float32)
    x = np.asarray(cut_coordinates, np.float32)
    goi = np.asarray(genes_oi).astype(np.int64)
    r = np.asarray(cut_local_cellxgene_ix).astype(np.int64)
    ix2 = np.asarray(cut_localcellxgene_ix).astype(np.int64)
    j = np.asarray(cut_local_gene_ix).astype(np.int32)
    hsw = np.asarray(height_slope_w, np.float32)
    osw = np.asarray(overall_slope_w, np.float32)
    obase = np.asarray(overall_baseline, np.float32)
    sbase = np.asarray(spline_baseline, np.float32)
    n_cuts = x.shape[0]

    # spline bin / frac exactly as the reference computes them (f32)
    xs = np.clip(x, np.float32(0.0), np.float32(1.0 - 1e-6)) * np.float32(K)
    b = np.clip(np.floor(xs).astype(np.int32), 0, K - 1)
    alpha = (xs - b.astype(np.float32)).astype(np.float32)

    core = (r // RPC).astype(np.int64)
    r_loc = (r - core * RPC).astype(np.int32)
    half = (r_loc >= HALF).astype(np.int64)

    # bucket grid shared by all cores: 256 buckets (half, b) per core
    NB = 2 * K
    key = core * NB + half * K + b
    cnt = np.bincount(key, minlength=N_CORES * NB).reshape(N_CORES, NB)
    slots_b = (cnt.max(axis=0) + SLOT - 1) // SLOT          # [256]
    slots_b = np.maximum(slots_b, 1)
    # half-0 slot region rounded up to a gather-group boundary
    h0 = int(slots_b[:K].sum())
    h0r = ((h0 + GS - 1) // GS) * GS
    h1 = int(slots_b[K:].sum())
    h1r = ((h1 + GS - 1) // GS) * GS
    off_b = np.zeros(NB + 1, np.int64)
    off_b[1:K + 1] = np.cumsum(slots_b[:K])
    off_b[K + 1:] = h0r + np.cumsum(slots_b[K:])
    # bucket slot ranges; extend last bucket of each half over region padding
    starts = off_b[:NB].copy()
    starts[K] = h0r
    ends = off_b[1:].copy()
    ends[K - 1] = h0r
    ends[NB - 1] = h0r + h1r
    T_pad = h0r + h1r
    G = T_pad // GS
    half_of_group = [0 if g * GS < h0r else 1 for g in range(G)]

    order = np.argsort(key, kind="stable")
    key_s = key[order]
    bucket_start = np.searchsorted(key_s, np.arange(N_CORES * NB))
    rank = np.arange(n_cuts) - bucket_start[key_s]
    bloc = key_s % NB
    slot = starts[bloc] + rank // SLOT
    part = rank % SLOT
    core_s = key_s // NB

    flat = core_s * (SLOT * T_pad) + part * T_pad + slot
    g1o = np.zeros(N_CORES * SLOT * T_pad, np.int16)
    g2o = np.zeros(N_CORES * SLOT * T_pad, np.int16)
    alf = np.zeros(N_CORES * SLOT * T_pad, np.float32)
    msk = np.zeros(N_CORES * SLOT * T_pad, np.float32)
    g1o[flat] = (r_loc[order] - (bloc >= K) * HALF).astype(np.int16)
    g2o[flat] = j[order].astype(np.int16)
    alf[flat] = alpha[order]
    msk[flat] = 1.0
    g1o = g1o.reshape(N_CORES, SLOT, T_pad)
    g2o = g2o.reshape(N_CORES, SLOT, T_pad)
    alf = alf.reshape(N_CORES, SLOT, T_pad)
    msk = msk.reshape(N_CORES, SLOT, T_pad)

    # wrapped int16 index streams for dma_gather:
    # element e (= slot*128 + part within a group) at [16*blk + e%16, e//16]
    def wrap_idx(a):  # a: [SLOT, T_pad] (partition, slot)
        e = np.ascontiguousarray(a.T).reshape(G, GS * SLOT)   # [G, 8192] e-major
        w = e.reshape(G, GC // 16, 16).transpose(0, 2, 1)     # [G, 16, 512]
        w = np.broadcast_to(w[:, None], (G, 8, 16, GC // 16))
        return np.ascontiguousarray(
            w.transpose(1, 2, 0, 3).reshape(SLOT, G * (GC // 16)))

    # per-gene params (small, replicated)
    W_oi = hsw[goi]                                          # [500, 10, 129]
    woiT = np.ascontiguousarray(
        W_oi.transpose(1, 0, 2).reshape(NL, N_GOI * NK)).astype(np.float32)
    ctab = np.zeros((N_GOI, ES), BF16)
    ctab[:, :NK] = sbase[goi].astype(BF16)
    oswT = np.concatenate([osw.T, obase[None, :]], axis=0).astype(np.float32)

    counts = np.bincount(ix2, minlength=N_CELLS * N_GT).reshape(N_CELLS, N_GT)
    cmax = counts.max()
    assert cmax < 256, f"count overflow {cmax}"
    counts = counts.astype(np.uint8)

    latw = np.concatenate(
        [latent.T, np.ones((1, N_CELLS), np.float32)], axis=0)  # [11, 1000]

    in_maps = []
    for kcore in range(N_CORES):
        in_maps.append({
            "latw": np.ascontiguousarray(latw[:, kcore * CPC:(kcore + 1) * CPC]),
            "woiT": woiT,
            "oswT": oswT,
            "ctab": ctab,
            "counts": np.ascontiguousarray(
                counts[kcore * CPC:(kcore + 1) * CPC]),
            "g1w": wrap_idx(g1o[kcore]),
            "g2w": wrap_idx(g2o[kcore]),
            "alpha": np.ascontiguousarray(alf[kcore]),
            "mask": np.ascontiguousarray(msk[kcore]),
        })
    grid = (tuple(int(s) for s in starts), tuple(int(e) for e in ends),
            int(G), int(T_pad), tuple(half_of_group))
    return in_maps, grid, n_cuts


def _build_program(starts, ends, G, T_pad, half_of_group,
                   phases="ABC", b_variant="full", iters=1):
    import concourse.bacc as bacc
    import concourse.bass as bass
    import concourse.mybir as mybir
    import concourse.tile as tile

    f32 = mybir.dt.float32
    bf16 = mybir.dt.bfloat16
    i16 = mybir.dt.int16
    u8 = mybir.dt.uint8
    Alu = mybir.AluOpType
    Act = mybir.ActivationFunctionType
    Ax = mybir.AxisListType
    NB = 2 * K
    IW = GC // 16                    # idx cols per group = 512

    nc = bacc.Bacc(None, target_bir_lowering=False)

    latw = nc.dram_tensor("latw", [NL + 1, CPC], f32, kind="ExternalInput")
    woiT = nc.dram_tensor("woiT", [NL, N_GOI * NK], f32, kind="ExternalInput")
    oswT = nc.dram_tensor("oswT", [NL + 1, N_GT], f32, kind="ExternalInput")
    ctab = nc.dram_tensor("ctab", [N_GOI, ES], bf16, kind="ExternalInput")
    counts = nc.dram_tensor("counts", [CPC, N_GT], u8, kind="ExternalInput")
    g1w_d = nc.dram_tensor("g1w", [SLOT, G * IW], i16, kind="ExternalInput")
    g2w_d = nc.dram_tensor("g2w", [SLOT, G * IW], i16, kind="ExternalInput")
    alpha_d = nc.dram_tensor("alpha", [SLOT, T_pad], f32, kind="ExternalInput")
    mask_d = nc.dram_tensor("mask", [SLOT, T_pad], f32, kind="ExternalInput")
    out_d = nc.dram_tensor("out", [2, 1], f32, kind="ExternalOutput")

    with tile.TileContext(nc) as tc:
        with (
            tc.tile_pool(name="dram", bufs=1, space="DRAM") as dpool,
            tc.tile_pool(name="outer", bufs=1) as lpool,
            tc.tile_pool(name="psum", bufs=4, space="PSUM") as ppool,
        ):
            A_tab = dpool.tile([RPC, ES], bf16)
            A_w = A_tab[:].rearrange("(c g) e -> c (g e)", c=CPC)  # [125, 500*256]

            latw_sb = lpool.tile([NL + 1, CPC], f32)
            nc.sync.dma_start(latw_sb[:], latw[:])
            accg = lpool.tile([SLOT, G], f32)
            nc.vector.memset(accg[:], 0.0)
            ovacc = lpool.tile([SLOT, 1], f32)
            nc.vector.memset(ovacc[:], 0.0)

            for _it in range(iters):
                # ---------------- Phase A: build the log-height table ----------
                GCH = 20                 # genes per staging chunk
                if "A" in phases:
                  with tc.tile_pool(name="build", bufs=3) as bpool:
                    for g0 in range(0, N_GOI, GCH):
                        ng = min(GCH, N_GOI - g0)
                        w = ng * NK
                        woi_sb = bpool.tile([NL, GCH * NK], f32, tag="woi")
                        if b_variant == "amm":
                            nc.vector.memset(woi_sb[:, :w], 0.1)
                        else:
                            nc.sync.dma_start(woi_sb[:, :w],
                                              woiT[:, g0 * NK:g0 * NK + w])
                        stag = bpool.tile([CPC, GCH * NK], bf16, tag="stag")
                        if b_variant == "adma":
                            nc.vector.memset(stag[:, :w], 0.1)
                        sub = 0
                        while sub < (0 if b_variant == "adma" else w):
                            sw = min(512, w - sub)
                            ps = ppool.tile([CPC, 512], f32, tag="ps")
                            nc.tensor.matmul(
                                out=ps[:, :sw],
                                lhsT=latw_sb[0:NL, :],
                                rhs=woi_sb[:, sub:sub + sw],
                                start=True, stop=True)
                            nc.vector.tensor_copy(stag[:, sub:sub + sw], ps[:, :sw])
                            sub += sw
                        # scatter 129-elem rows into the 256-elem padded layout
                        if b_variant == "anodma":
                            pass
                        elif b_variant == "acontig":
                            nc.sync.dma_start(
                                A_w[:, g0 * ES:g0 * ES + w], stag[:, :w])
                        else:
                            dst = A_w[:, g0 * ES:(g0 + ng) * ES].rearrange(
                                "c (g e) -> c g e", e=ES)[:, :, 0:NK]
                            src = stag[:, :w].rearrange("c (g e) -> c g e", e=NK)
                            nc.sync.dma_start(dst, src)

                # ---------------- Phase C: overall (softmax) term --------------
                if "C" in phases:
                  with tc.tile_pool(name="ovp", bufs=1) as opool:
                    osw_sb = opool.tile([NL + 1, N_GT], f32)
                    nc.sync.dma_start(osw_sb[:], oswT[:])
                    scores = opool.tile([CPC, N_GT], f32)
                    sub = 0
                    while sub < N_GT:
                        sw = min(512, N_GT - sub)
                        ps = ppool.tile([CPC, 512], f32, tag="ps")
                        nc.tensor.matmul(
                            out=ps[:, :sw],
                            lhsT=latw_sb[:, :],
                            rhs=osw_sb[:, sub:sub + sw],
                            start=True, stop=True)
                        nc.vector.tensor_copy(scores[:, sub:sub + sw], ps[:, :sw])
                        sub += sw
                    mrow = opool.tile([CPC, 1], f32)
                    nc.vector.tensor_reduce(mrow[:], scores[:], axis=Ax.X, op=Alu.max)
                    negm = opool.tile([CPC, 1], f32)
                    nc.vector.tensor_scalar_mul(negm[:], mrow[:], -1.0)
                    etrash = opool.tile([CPC, N_GT], bf16)
                    sume = opool.tile([CPC, 1], f32)
                    nc.scalar.activation(etrash[:], scores[:], Act.Exp,
                                         bias=negm[:], scale=1.0,
                                         accum_out=sume[:])
                    lnse = opool.tile([CPC, 1], f32)
                    nc.scalar.activation(lnse[:], sume[:], Act.Ln)
                    lse = opool.tile([CPC, 1], f32)
                    nc.vector.tensor_tensor(out=lse[:], in0=mrow[:], in1=lnse[:],
                                            op=Alu.add)
                    cts_sb = opool.tile([CPC, N_GT], u8)
                    nc.sync.dma_start(cts_sb[:], counts[:])
                    ctsf = opool.tile([CPC, N_GT], f32)
                    nc.vector.tensor_copy(ctsf[:], cts_sb[:])
                    nc.vector.scalar_tensor_tensor(
                        out=scores[:], in0=scores[:], scalar=lse[:], in1=ctsf[:],
                        op0=Alu.subtract, op1=Alu.mult,
                        accum_out=ovacc[0:CPC, :])

                # ---------------- Phase B: per-cut spline likelihood -----------
                with tc.tile_pool(name="main", bufs=2) as mpool:
                    for g in range(G if "B" in phases else 0):
                        s0, s1 = g * GS, (g + 1) * GS
                        hf = half_of_group[g]
                        al_sb = mpool.tile([SLOT, GS], f32, tag="al")
                        nc.sync.dma_start(al_sb[:], alpha_d[:, s0:s1])
                        mk_sb = mpool.tile([SLOT, GS], f32, tag="mk")
                        nc.sync.dma_start(mk_sb[:], mask_d[:, s0:s1])
                        i1_sb = mpool.tile([SLOT, IW], i16, tag="i1")
                        nc.sync.dma_start(i1_sb[:], g1w_d[:, g * IW:(g + 1) * IW])
                        i2_sb = mpool.tile([SLOT, IW], i16, tag="i2")
                        nc.sync.dma_start(i2_sb[:], g2w_d[:, g * IW:(g + 1) * IW])

                        ha = mpool.tile([SLOT, GS, ES], bf16, tag="ha")
                        if b_variant == "none":
                            nc.vector.memset(ha[:], 0.5)
                        else:
                            nc.gpsimd.dma_gather(
                                out_ap=ha[:],
                                in_ap=A_tab[hf * HALF:hf * HALF + HALF, :],
                                idxs_ap=i1_sb[:], num_idxs=GC, num_idxs_reg=GC,
                                elem_size=ES, single_packet=False)
                        if b_variant == "g1":
                            nc.vector.tensor_reduce(accg[:, g:g + 1],
                                                    ha[:, :, 0:NK],
                                                    axis=Ax.XY, op=Alu.add)
                            continue
                        hc = mpool.tile([SLOT, GS, ES], bf16, tag="hc")
                        if b_variant == "none":
                            nc.vector.memset(hc[:], 0.5)
                        else:
                            nc.gpsimd.dma_gather(
                                out_ap=hc[:], in_ap=ctab[:],
                                idxs_ap=i2_sb[:], num_idxs=GC, num_idxs_reg=GC,
                                elem_size=ES, single_packet=False)
                        nc.vector.tensor_tensor(
                            out=ha[:, :, 0:NK], in0=ha[:, :, 0:NK],
                            in1=hc[:, :, 0:NK], op=Alu.add)
                        if b_variant == "g1g2":
                            nc.vector.tensor_reduce(accg[:, g:g + 1],
                                                    ha[:, :, 0:NK],
                                                    axis=Ax.XY, op=Alu.add)
                            continue

                        nc.scalar.activation(ha[:, :, 0:NK], ha[:, :, 0:NK],
                                             Act.Exp)   # u = exp(h)
                        if b_variant == "exp":
                            nc.vector.tensor_reduce(accg[:, g:g + 1],
                                                    ha[:, :, 0:NK],
                                                    axis=Ax.XY, op=Alu.add)
                            continue

                        S0t = mpool.tile([SLOT, GS], f32, tag="S0")
                        nc.vector.tensor_reduce(S0t[:], ha[:, :, 0:NK],
                                                axis=Ax.X, op=Alu.add)
                        endst = mpool.tile([SLOT, GS], f32, tag="ends")
                        nc.vector.tensor_tensor(out=endst[:], in0=ha[:, :, 0],
                                                in1=ha[:, :, K], op=Alu.add)
                        Stt = mpool.tile([SLOT, GS], f32, tag="St")
                        nc.vector.scalar_tensor_tensor(
                            out=Stt[:], in0=endst[:], scalar=-0.5, in1=S0t[:],
                            op0=Alu.mult, op1=Alu.add)

                        pr = mpool.tile([SLOT, GS, 2], f32, tag="pr")
                        for bb in range(NB):
                            lo = max(starts[bb], s0)
                            hi = min(ends[bb], s1)
                            if lo >= hi:
                                continue
                            col = bb % K
                            nc.vector.tensor_copy(
                                pr[:, lo - s0:hi - s0, :],
                                ha[:, lo - s0:hi - s0, col:col + 2])

                        dt_ = mpool.tile([SLOT, GS], f32, tag="dt")
                        nc.vector.tensor_tensor(out=dt_[:], in0=pr[:, :, 1],
                                                in1=pr[:, :, 0], op=Alu.subtract)
                        t1 = mpool.tile([SLOT, GS], f32, tag="t1")
                        nc.vector.tensor_tensor(out=t1[:], in0=al_sb[:],
                                                in1=dt_[:], op=Alu.mult)
                        It = mpool.tile([SLOT, GS], f32, tag="It")
                        nc.vector.tensor_tensor(out=It[:], in0=t1[:],
                                                in1=pr[:, :, 0], op=Alu.add)
                        logI = mpool.tile([SLOT, GS], f32, tag="logI")
                        nc.scalar.activation(logI[:], It[:], Act.Ln)
                        logS = mpool.tile([SLOT, GS], f32, tag="logS")
                        nc.scalar.activation(logS[:], Stt[:], Act.Ln)
                        lik = mpool.tile([SLOT, GS], f32, tag="lik")
                        nc.vector.tensor_tensor(out=lik[:], in0=logI[:],
                                                in1=logS[:], op=Alu.subtract)
                        mlik = mpool.tile([SLOT, GS], f32, tag="mlik")
                        nc.vector.tensor_tensor(out=mlik[:], in0=lik[:],
                                                in1=mk_sb[:], op=Alu.mult)
                        nc.vector.tensor_reduce(accg[:, g:g + 1], mlik[:],
                                                axis=Ax.X, op=Alu.add)

            # -------- final reduction to two scalars --------
            acc1 = lpool.tile([SLOT, 1], f32)
            nc.vector.tensor_reduce(acc1[:], accg[:], axis=Ax.X, op=Alu.add)
            comb = lpool.tile([SLOT, 2], f32)
            nc.vector.memset(comb[:], 0.0)
            nc.vector.tensor_copy(comb[:, 0:1], acc1[:])
            nc.vector.tensor_copy(comb[:, 1:2], ovacc[:])
            ones = lpool.tile([SLOT, 1], f32)
            nc.vector.memset(ones[:], 1.0)
            pres = ppool.tile([2, 1], f32, tag="pres")
            nc.tensor.matmul(out=pres[:], lhsT=comb[:], rhs=ones[:],
                             start=True, stop=True)
            res_sb = lpool.tile([2, 1], f32)
            nc.vector.tensor_copy(res_sb[:], pres[:])
            nc.sync.dma_start(out_d[:], res_sb[:])

    nc.finalize()
    return nc


def kernel(**inputs) -> np.ndarray:
    from concourse.bass_utils import run_bass_kernel_spmd

    in_maps, grid, n_cuts = _host_prep(**inputs)
    if grid in _PROGRAM_CACHE:
        nc = _PROGRAM_CACHE[grid]
    else:
        nc = _build_program(*grid)
        _PROGRAM_CACHE[grid] = nc

    res = run_bass_kernel_spmd(nc, in_maps, list(range(N_CORES)))
    total = 0.0
    for kcore in range(N_CORES):
        o = np.asarray(res.results[kcore]["out"], np.float64)
        total += o[0, 0] + o[1, 0]
    total += n_cuts * (np.log(128.0) + np.log(5000.0))
    return np.float32(-total)



# revision 2
# speedup vs baseline: 4.7704x; 4.7704x over previous
"""Trainium2 Bass kernel for nn_Decoding_33019708572164 (ragged spline decoder ELBO).

Strategy (8 NeuronCores, data-parallel over the 1M ragged cuts, transfer-lean):
  - Cuts are routed to cores by the GENE of their height-row index
    r = cut_local_cellxgene_ix (core = (r % 500) % 8), so each core only needs
    its 63-gene slice of height_slope_w (bf16) instead of the full tensor.
  - Exp is factorized out of the spline: u_k = exp(sbase_k + A_k)
    = EC[j,k] * EA[r,k].  Phase A builds EA = exp(latent . w) as a bf16 DRAM
    table (rows padded to 256 elems for dma_gather); EC = exp(sbase[genes_oi])
    is computed on host (tiny) and padded on device.
  - Padding cuts index special all-ones rows of both tables, which makes their
    likelihood exactly log(1) - log(128); the host subtracts that analytically,
    so no mask array is shipped or applied.
  - Per-cut data shipped per core: two compact i16 gather-index streams
    ([16, G*512], replicated across the 8 DSP-core partition blocks on device),
    and a u8-quantized alpha.  Per-gene params are bf16; the phase-C count
    histogram is packed two 4-bit counts per byte. ~1.5MB/core vs ~10.5MB
    before - the axon-tunneled host->device transfer is the dominant cost.
  - Phase B per 8192-cut group: 2 dma_gathers, u = EA*EC (bf16, in-place),
    trapezoid norm, static-column interpolation per (half, bin) bucket,
    Ln with accum_out.
  - Phase C: sum(counts * log_softmax) with counts = host bincount of
    cut_localcellxgene_ix; each core handles a 125-cell slab.
"""

import sys

if "/opt/trn_rl_repo" not in sys.path:
    sys.path.insert(0, "/opt/trn_rl_repo")

import numpy as np
import ml_dtypes

N_CORES = 8
N_CELLS = 1000
N_GOI = 500
N_GT = 5000
NL = 10
K = 128
NK = 129
ES = 256                          # padded row length (bf16) = 512B
GPC = 63                          # gene slots per core (mod-8 sharding)
RPC = N_CELLS * GPC               # table rows per core = 63000
HALF = RPC // 2                   # 31500 real rows per half (int16 idx range)
HALFP = HALF + 1                  # plus the all-ones pad row
CPC = N_CELLS // N_CORES          # phase-C cells per core = 125
SLOT = 128                        # cuts per slot (partition dim)
GS = 64                           # slots per gather group (8192 cuts)
GC = GS * SLOT                    # cuts per group
IW = GC // 16                     # idx cols per group = 512
BF16 = ml_dtypes.bfloat16

_PROGRAM_CACHE = {}


def _host_prep(latent, cut_coordinates, genes_oi, cut_local_cellxgene_ix,
               cut_localcellxgene_ix, cut_local_gene_ix, height_slope_w,
               overall_slope_w, overall_baseline, spline_baseline):
    latent = np.asarray(latent, np.float32)
    x = np.asarray(cut_coordinates, np.float32)
    goi = np.asarray(genes_oi).astype(np.int64)
    r = np.asarray(cut_local_cellxgene_ix).astype(np.int64)
    ix2 = np.asarray(cut_localcellxgene_ix).astype(np.int64)
    j = np.asarray(cut_local_gene_ix).astype(np.int32)
    hsw = np.asarray(height_slope_w, np.float32)
    osw = np.asarray(overall_slope_w, np.float32)
    obase = np.asarray(overall_baseline, np.float32)
    sbase = np.asarray(spline_baseline, np.float32)
    n_cuts = x.shape[0]

    # spline bin / frac exactly as the reference computes them (f32)
    xs = np.clip(x, np.float32(0.0), np.float32(1.0 - 1e-6)) * np.float32(K)
    b = np.clip(np.floor(xs).astype(np.int32), 0, K - 1)
    alpha = (xs - b.astype(np.float32)).astype(np.float32)
    aq = np.minimum(np.floor(alpha * np.float32(256.0)), 255).astype(np.uint8)

    gene_r = (r % N_GOI).astype(np.int64)
    cell_r = r // N_GOI
    core = gene_r % N_CORES
    gl = gene_r // N_CORES                                  # [0, 63)
    r_loc = (cell_r * GPC + gl).astype(np.int32)            # [0, 63000)

    # bucket grid shared by all cores: 256 buckets (half, b) per core
    NB = 2 * K
    half = (r_loc >= HALF).astype(np.int64)
    key = core * NB + half * K + b
    cnt = np.bincount(key, minlength=N_CORES * NB).reshape(N_CORES, NB)
    slots_b = (cnt.max(axis=0) + SLOT - 1) // SLOT          # [256]
    slots_b = np.maximum(slots_b, 1)
    h0 = int(slots_b[:K].sum())
    h0r = ((h0 + GS - 1) // GS) * GS
    h1 = int(slots_b[K:].sum())
    h1r = ((h1 + GS - 1) // GS) * GS
    off_b = np.zeros(NB + 1, np.int64)
    off_b[1:K + 1] = np.cumsum(slots_b[:K])
    off_b[K + 1:] = h0r + np.cumsum(slots_b[K:])
    # bucket slot ranges; extend last bucket of each half over region padding
    starts = off_b[:NB].copy()
    starts[K] = h0r
    ends = off_b[1:].copy()
    ends[K - 1] = h0r
    ends[NB - 1] = h0r + h1r
    T_pad = h0r + h1r
    G = T_pad // GS
    half_of_group = [0 if g * GS < h0r else 1 for g in range(G)]

    order = np.argsort(key, kind="stable")
    key_s = key[order]
    bucket_start = np.searchsorted(key_s, np.arange(N_CORES * NB))
    rank = np.arange(n_cuts) - bucket_start[key_s]
    bloc = key_s % NB
    slot = starts[bloc] + rank // SLOT
    part = rank % SLOT
    core_s = key_s // NB

    flat = core_s * (SLOT * T_pad) + part * T_pad + slot
    g1o = np.full(N_CORES * SLOT * T_pad, HALF, np.int16)   # pad -> ones row
    g2o = np.full(N_CORES * SLOT * T_pad, N_GOI, np.int16)  # pad -> ones row
    al8 = np.zeros(N_CORES * SLOT * T_pad, np.uint8)
    g1o[flat] = (r_loc[order] - (bloc >= K) * HALF).astype(np.int16)
    g2o[flat] = j[order].astype(np.int16)
    al8[flat] = aq[order]
    g1o = g1o.reshape(N_CORES, SLOT, T_pad)
    g2o = g2o.reshape(N_CORES, SLOT, T_pad)
    al8 = al8.reshape(N_CORES, SLOT, T_pad)
    n_pad = N_CORES * SLOT * T_pad - n_cuts

    # compact int16 index streams for dma_gather: element e (= slot*128 + part
    # within a group) at [e%16, e//16]; the 8x partition-block replication the
    # engine wants is done on device.
    def wrap_idx(a):  # a: [SLOT, T_pad] (partition, slot)
        e = np.ascontiguousarray(a.T).reshape(G, GC)          # [G, 8192]
        w = e.reshape(G, GC // 16, 16).transpose(0, 2, 1)     # [G, 16, 512]
        return np.ascontiguousarray(w.transpose(1, 0, 2).reshape(16, G * IW))

    # per-gene params
    W_oi = hsw[goi]                                          # [500, 10, 129]
    ec = np.ones((N_GOI + 1, NK), np.float32)
    ec[:N_GOI] = np.exp(sbase[goi])
    ec = ec.astype(BF16)
    oswT = np.concatenate([osw.T, obase[None, :]], axis=0).astype(BF16)
    latw = np.concatenate(
        [latent.T, np.ones((1, N_CELLS), np.float32)], axis=0).astype(BF16)

    counts = np.bincount(ix2, minlength=N_CELLS * N_GT).reshape(N_CELLS, N_GT)
    cmax = counts.max()
    assert cmax < 16, f"count overflow {cmax}"
    counts = counts.astype(np.uint8)

    in_maps = []
    for kcore in range(N_CORES):
        sel = np.arange(kcore, N_GOI, N_CORES)
        wk = np.zeros((GPC, NL, NK), np.float32)
        wk[:len(sel)] = W_oi[sel]
        woiT = np.ascontiguousarray(
            wk.transpose(1, 0, 2).reshape(NL, GPC * NK)).astype(BF16)
        cslab = counts[kcore * CPC:(kcore + 1) * CPC]
        cpk = (cslab[:, :N_GT // 2] | (cslab[:, N_GT // 2:] << 4)).astype(np.uint8)
        in_maps.append({
            "latw": latw,
            "lown": np.ascontiguousarray(latw[:, kcore * CPC:(kcore + 1) * CPC]),
            "woiT": woiT,
            "oswT": oswT,
            "ectab": ec,
            "cpk": cpk,
            "g1w": wrap_idx(g1o[kcore]),
            "g2w": wrap_idx(g2o[kcore]),
            "al8": np.ascontiguousarray(al8[kcore]),
        })
    grid = (tuple(int(s) for s in starts), tuple(int(e) for e in ends),
            int(G), int(T_pad), tuple(half_of_group))
    return in_maps, grid, n_cuts, n_pad


def _build_program(starts, ends, G, T_pad, half_of_group,
                   phases="ABC", b_variant="full"):
    import concourse.bacc as bacc
    import concourse.bass as bass
    import concourse.mybir as mybir
    import concourse.tile as tile

    f32 = mybir.dt.float32
    bf16 = mybir.dt.bfloat16
    i16 = mybir.dt.int16
    u8 = mybir.dt.uint8
    Alu = mybir.AluOpType
    Act = mybir.ActivationFunctionType
    Ax = mybir.AxisListType
    NB = 2 * K
    WW = GPC * NK                    # 8127 table cols per cell

    nc = bacc.Bacc(None, target_bir_lowering=False)

    latw = nc.dram_tensor("latw", [NL + 1, N_CELLS], bf16, kind="ExternalInput")
    lown = nc.dram_tensor("lown", [NL + 1, CPC], bf16, kind="ExternalInput")
    woiT = nc.dram_tensor("woiT", [NL, WW], bf16, kind="ExternalInput")
    oswT = nc.dram_tensor("oswT", [NL + 1, N_GT], bf16, kind="ExternalInput")
    ec_d = nc.dram_tensor("ectab", [N_GOI + 1, NK], bf16, kind="ExternalInput")
    cpk_d = nc.dram_tensor("cpk", [CPC, N_GT // 2], u8, kind="ExternalInput")
    g1w_d = nc.dram_tensor("g1w", [16, G * IW], i16, kind="ExternalInput")
    g2w_d = nc.dram_tensor("g2w", [16, G * IW], i16, kind="ExternalInput")
    al8_d = nc.dram_tensor("al8", [SLOT, T_pad], u8, kind="ExternalInput")
    out_d = nc.dram_tensor("out", [2, 1], f32, kind="ExternalOutput")

    with tile.TileContext(nc) as tc:
        with (
            tc.tile_pool(name="dram", bufs=1, space="DRAM") as dpool,
            tc.tile_pool(name="outer", bufs=1) as lpool,
            tc.tile_pool(name="psum", bufs=4, space="PSUM") as ppool,
        ):
            A_tab = dpool.tile([2 * HALFP, ES], bf16)
            ECT = dpool.tile([N_GOI + 1, ES], bf16)

            latw_sb = lpool.tile([NL + 1, N_CELLS], bf16)
            nc.sync.dma_start(latw_sb[:], latw[:])
            lown_sb = lpool.tile([NL + 1, CPC], bf16)
            nc.sync.dma_start(lown_sb[:], lown[:])
            accI = lpool.tile([SLOT, G], f32)
            nc.vector.memset(accI[:], 0.0)
            accS = lpool.tile([SLOT, G], f32)
            nc.vector.memset(accS[:], 0.0)
            ovacc = lpool.tile([SLOT, 2], f32)
            nc.vector.memset(ovacc[:], 0.0)
            i1_all = lpool.tile([SLOT, G * IW], i16)
            i2_all = lpool.tile([SLOT, G * IW], i16)
            for b8 in range(8):
                nc.sync.dma_start(i1_all[16 * b8:16 * (b8 + 1), :], g1w_d[:])
                nc.sync.dma_start(i2_all[16 * b8:16 * (b8 + 1), :], g2w_d[:])
            al_all = lpool.tile([SLOT, T_pad], u8)
            nc.sync.dma_start(al_all[:], al8_d[:])

            # EC table: pad 129-elem rows into the 256-elem gather layout
            nc.sync.dma_start(ECT[:, 0:NK], ec_d[:])
            ones_sb = lpool.tile([2, ES], bf16)
            nc.vector.memset(ones_sb[:], 1.0)
            nc.sync.dma_start(A_tab[HALF:HALF + 1, :], ones_sb[0:1, :])
            nc.sync.dma_start(A_tab[2 * HALFP - 1:2 * HALFP, :], ones_sb[1:2, :])

            # ---------------- Phase A: build EA = exp(latent . w) ----------
            if "A" in phases:
              with tc.tile_pool(name="build", bufs=3) as bpool:
                woi_sb = bpool.tile([NL, WW], bf16, tag="woi")
                nc.sync.dma_start(woi_sb[:], woiT[:])
                for t in range(8):
                    stag = bpool.tile([CPC, WW], bf16, tag="stag")
                    sub = 0
                    while sub < WW:
                        sw = min(512, WW - sub)
                        ps = ppool.tile([CPC, 512], f32, tag="ps")
                        nc.tensor.matmul(
                            out=ps[:, :sw],
                            lhsT=latw_sb[0:NL, t * CPC:(t + 1) * CPC],
                            rhs=woi_sb[:, sub:sub + sw],
                            start=True, stop=True)
                        nc.scalar.activation(stag[:, sub:sub + sw], ps[:, :sw],
                                             Act.Exp)
                        sub += sw
                    r0 = t * CPC * GPC if t < 4 else HALFP + (t - 4) * CPC * GPC
                    dst = A_tab[r0:r0 + CPC * GPC, :].rearrange(
                        "(c g) e -> c g e", c=CPC)[:, :, 0:NK]
                    src = stag[:].rearrange("c (g e) -> c g e", e=NK)
                    nc.sync.dma_start(dst, src)

            # ---------------- Phase C: overall (softmax) term --------------
            if "C" in phases:
              with tc.tile_pool(name="ovp", bufs=1) as opool:
                osw_sb = opool.tile([NL + 1, N_GT], bf16)
                nc.sync.dma_start(osw_sb[:], oswT[:])
                scores = opool.tile([CPC, N_GT], f32)
                sub = 0
                while sub < N_GT:
                    sw = min(512, N_GT - sub)
                    ps = ppool.tile([CPC, 512], f32, tag="ps")
                    nc.tensor.matmul(
                        out=ps[:, :sw],
                        lhsT=lown_sb[:, :],
                        rhs=osw_sb[:, sub:sub + sw],
                        start=True, stop=True)
                    nc.vector.tensor_copy(scores[:, sub:sub + sw], ps[:, :sw])
                    sub += sw
                mrow = opool.tile([CPC, 1], f32)
                nc.vector.tensor_reduce(mrow[:], scores[:], axis=Ax.X, op=Alu.max)
                negm = opool.tile([CPC, 1], f32)
                nc.vector.tensor_scalar_mul(negm[:], mrow[:], -1.0)
                etrash = opool.tile([CPC, N_GT], bf16)
                sume = opool.tile([CPC, 1], f32)
                nc.scalar.activation(etrash[:], scores[:], Act.Exp,
                                     bias=negm[:], scale=1.0,
                                     accum_out=sume[:])
                lnse = opool.tile([CPC, 1], f32)
                nc.scalar.activation(lnse[:], sume[:], Act.Ln)
                lse = opool.tile([CPC, 1], f32)
                nc.vector.tensor_tensor(out=lse[:], in0=mrow[:], in1=lnse[:],
                                        op=Alu.add)
                cp_sb = opool.tile([CPC, N_GT // 2], u8)
                nc.sync.dma_start(cp_sb[:], cpk_d[:])
                lo8 = opool.tile([CPC, N_GT // 2], u8)
                nc.vector.tensor_scalar(out=lo8[:], in0=cp_sb[:], scalar1=15,
                                        scalar2=None, op0=Alu.bitwise_and)
                hi8 = opool.tile([CPC, N_GT // 2], u8)
                nc.vector.tensor_scalar(out=hi8[:], in0=cp_sb[:], scalar1=4,
                                        scalar2=None,
                                        op0=Alu.logical_shift_right)
                clo = opool.tile([CPC, N_GT // 2], f32)
                nc.vector.tensor_copy(clo[:], lo8[:])
                chi = opool.tile([CPC, N_GT // 2], f32)
                nc.vector.tensor_copy(chi[:], hi8[:])
                nc.vector.scalar_tensor_tensor(
                    out=scores[:, :N_GT // 2], in0=scores[:, :N_GT // 2],
                    scalar=lse[:], in1=clo[:],
                    op0=Alu.subtract, op1=Alu.mult,
                    accum_out=ovacc[0:CPC, 0:1])
                nc.vector.scalar_tensor_tensor(
                    out=scores[:, N_GT // 2:], in0=scores[:, N_GT // 2:],
                    scalar=lse[:], in1=chi[:],
                    op0=Alu.subtract, op1=Alu.mult,
                    accum_out=ovacc[0:CPC, 1:2])

            # ---------------- Phase B: per-cut spline likelihood -----------
            with tc.tile_pool(name="main", bufs=2) as mpool:
                for g in range(G if "B" in phases else 0):
                    s0, s1 = g * GS, (g + 1) * GS
                    hf = half_of_group[g]
                    ha = mpool.tile([SLOT, GS, ES], bf16, tag="ha")
                    if b_variant == "none":
                        nc.vector.memset(ha[:], 0.5)
                    else:
                        nc.gpsimd.dma_gather(
                            out_ap=ha[:],
                            in_ap=A_tab[hf * HALFP:(hf + 1) * HALFP, :],
                            idxs_ap=i1_all[:, g * IW:(g + 1) * IW],
                            num_idxs=GC, num_idxs_reg=GC,
                            elem_size=ES, single_packet=False)
                    if b_variant == "g1":
                        nc.vector.tensor_reduce(accI[:, g:g + 1],
                                                ha[:, :, 0:NK],
                                                axis=Ax.XY, op=Alu.add)
                        continue
                    hc = mpool.tile([SLOT, GS, ES], bf16, tag="hc")
                    if b_variant == "none":
                        nc.vector.memset(hc[:], 0.5)
                    else:
                        nc.gpsimd.dma_gather(
                            out_ap=hc[:], in_ap=ECT[:],
                            idxs_ap=i2_all[:, g * IW:(g + 1) * IW],
                            num_idxs=GC, num_idxs_reg=GC,
                            elem_size=ES, single_packet=False)
                    if b_variant == "g1g2":
                        nc.vector.tensor_reduce(accI[:, g:g + 1],
                                                ha[:, :, 0:NK],
                                                axis=Ax.XY, op=Alu.add)
                        continue
                    # u = EA * EC, in place (bf16)
                    nc.vector.tensor_tensor(
                        out=ha[:, :, 0:NK], in0=ha[:, :, 0:NK],
                        in1=hc[:, :, 0:NK], op=Alu.mult)

                    S0t = mpool.tile([SLOT, GS], f32, tag="S0")
                    nc.vector.tensor_reduce(S0t[:], ha[:, :, 0:NK],
                                            axis=Ax.X, op=Alu.add)
                    endst = mpool.tile([SLOT, GS], f32, tag="ends")
                    nc.vector.tensor_tensor(out=endst[:], in0=ha[:, :, 0],
                                            in1=ha[:, :, K], op=Alu.add)
                    Stt = mpool.tile([SLOT, GS], f32, tag="St")
                    nc.vector.scalar_tensor_tensor(
                        out=Stt[:], in0=endst[:], scalar=-0.5, in1=S0t[:],
                        op0=Alu.mult, op1=Alu.add)

                    pr = mpool.tile([SLOT, GS, 2], f32, tag="pr")
                    for bb in range(NB):
                        lo = max(starts[bb], s0)
                        hi = min(ends[bb], s1)
                        if lo >= hi:
                            continue
                        col = bb % K
                        nc.vector.tensor_copy(
                            pr[:, lo - s0:hi - s0, :],
                            ha[:, lo - s0:hi - s0, col:col + 2])

                    alf = mpool.tile([SLOT, GS], f32, tag="alf")
                    nc.vector.tensor_copy(alf[:], al_all[:, s0:s1])
                    alf2 = mpool.tile([SLOT, GS], f32, tag="alf2")
                    nc.vector.tensor_scalar(out=alf2[:], in0=alf[:],
                                            scalar1=0.5, scalar2=1.0 / 256.0,
                                            op0=Alu.add, op1=Alu.mult)
                    dt_ = mpool.tile([SLOT, GS], f32, tag="dt")
                    nc.vector.tensor_tensor(out=dt_[:], in0=pr[:, :, 1],
                                            in1=pr[:, :, 0], op=Alu.subtract)
                    t1 = mpool.tile([SLOT, GS], f32, tag="t1")
                    nc.vector.tensor_tensor(out=t1[:], in0=alf2[:],
                                            in1=dt_[:], op=Alu.mult)
                    It = mpool.tile([SLOT, GS], f32, tag="It")
                    nc.vector.tensor_tensor(out=It[:], in0=t1[:],
                                            in1=pr[:, :, 0], op=Alu.add)
                    lgI = mpool.tile([SLOT, GS], f32, tag="lgI")
                    nc.scalar.activation(lgI[:], It[:], Act.Ln,
                                         accum_out=accI[:, g:g + 1])
                    lgS = mpool.tile([SLOT, GS], f32, tag="lgS")
                    nc.scalar.activation(lgS[:], Stt[:], Act.Ln,
                                         accum_out=accS[:, g:g + 1])

            # -------- final reduction to two scalars --------
            accd = lpool.tile([SLOT, 1], f32)
            nc.vector.tensor_reduce(accd[:], accI[:], axis=Ax.X, op=Alu.add)
            accs1 = lpool.tile([SLOT, 1], f32)
            nc.vector.tensor_reduce(accs1[:], accS[:], axis=Ax.X, op=Alu.add)
            acc1 = lpool.tile([SLOT, 1], f32)
            nc.vector.tensor_tensor(out=acc1[:], in0=accd[:], in1=accs1[:],
                                    op=Alu.subtract)
            ovs = lpool.tile([SLOT, 1], f32)
            nc.vector.tensor_reduce(ovs[:], ovacc[:], axis=Ax.X, op=Alu.add)
            comb = lpool.tile([SLOT, 2], f32)
            nc.vector.tensor_copy(comb[:, 0:1], acc1[:])
            nc.vector.tensor_copy(comb[:, 1:2], ovs[:])
            ones1 = lpool.tile([SLOT, 1], f32)
            nc.vector.memset(ones1[:], 1.0)
            pres = ppool.tile([2, 1], f32, tag="pres")
            nc.tensor.matmul(out=pres[:], lhsT=comb[:], rhs=ones1[:],
                             start=True, stop=True)
            res_sb = lpool.tile([2, 1], f32)
            nc.vector.tensor_copy(res_sb[:], pres[:])
            nc.sync.dma_start(out_d[:], res_sb[:])

    nc.finalize()
    return nc


def kernel(**inputs) -> np.ndarray:
    from concourse.bass_utils import run_bass_kernel_spmd

    in_maps, grid, n_cuts, n_pad = _host_prep(**inputs)
    if grid in _PROGRAM_CACHE:
        nc = _PROGRAM_CACHE[grid]
    else:
        nc = _build_program(*grid)
        _PROGRAM_CACHE[grid] = nc

    res = run_bass_kernel_spmd(nc, in_maps, list(range(N_CORES)))
    total = 0.0
    for kcore in range(N_CORES):
        o = np.asarray(res.results[kcore]["out"], np.float64)
        total += o[0, 0] + o[1, 0]
    total += n_cuts * (np.log(128.0) + np.log(5000.0)) + n_pad * np.log(128.0)
    return np.float32(-total)


# revision 7
# speedup vs baseline: 5.7542x; 1.2062x over previous
"""Trainium2 Bass kernel for nn_Decoding_33019708572164 (ragged spline decoder ELBO).

Strategy (8 NeuronCores, data-parallel over the 1M ragged cuts, transfer-lean):
  - Cuts are routed to cores by the GENE of their height-row index
    r = cut_local_cellxgene_ix (core = (r % 500) % 8), so each core only needs
    its 63-gene slice of height_slope_w (bf16) instead of the full tensor.
  - Exp is factorized out of the spline: u_k = exp(sbase_k + A_k)
    = EC[j,k] * EA[r,k].  Phase A builds EA = exp(latent . w) as a bf16 DRAM
    table (rows padded to 256 elems for dma_gather); EC = exp(sbase[genes_oi])
    is computed on host (tiny) and padded on device.
  - Padding cuts index special all-ones rows of both tables, which makes their
    likelihood exactly log(1) - log(128); the host subtracts that analytically,
    so no mask array is shipped or applied.
  - Cuts are bucketed by (table half, spline double-bin b//2): 128 buckets.
    Interpolation uses three static columns [2m, 2m+1, 2m+2] per bucket and a
    per-cut position t = xs - 2m in [0,2), quantized to 7 bits and packed into
    the spare bits of the EC gather index (j needs 9 bits of the int16).
  - Per-cut data shipped per core: two compact i16 gather-index streams
    ([16, G*512], replicated across the 8 DSP-core partition blocks on device).
    The phase-C count histogram is packed four 2-bit counts per byte (counts
    are clipped at 3; the rare overflow is corrected analytically on host).
    ~1.15MB/core - the axon-tunneled host->device transfer dominates runtime.
  - Phase B per 8192-cut group: 2 dma_gathers, u = EA*EC (bf16, in-place),
    trapezoid norm, 3-column interpolation, Ln with accum_out.
  - Phase C: sum(counts * log_softmax) with counts = host bincount of
    cut_localcellxgene_ix; each core handles a 125-cell slab.
"""

import sys

if "/opt/trn_rl_repo" not in sys.path:
    sys.path.insert(0, "/opt/trn_rl_repo")

import numpy as np
import ml_dtypes

N_CORES = 8
N_CELLS = 1000
N_GOI = 500
N_GT = 5000
NL = 10
K = 128
NK = 129
M2 = K // 2                       # double-bins = 64
ES = 256                          # padded row length (bf16) = 512B
GPC = 63                          # gene slots per core (mod-8 sharding)
RPC = N_CELLS * GPC               # table rows per core = 63000
HALF = RPC // 2                   # 31500 real rows per half (int16 idx range)
HALFP = HALF + 1                  # plus the all-ones pad row
CPC = N_CELLS // N_CORES          # phase-C cells per core = 125
SLOT = 128                        # cuts per slot (partition dim)
GS = 64                           # slots per gather group (8192 cuts)
GC = GS * SLOT                    # cuts per group
IW = GC // 16                     # idx cols per group = 512
BF16 = ml_dtypes.bfloat16

_PROGRAM_CACHE = {}


def _host_prep(latent, cut_coordinates, genes_oi, cut_local_cellxgene_ix,
               cut_localcellxgene_ix, cut_local_gene_ix, height_slope_w,
               overall_slope_w, overall_baseline, spline_baseline):
    latent = np.asarray(latent, np.float32)
    x = np.asarray(cut_coordinates, np.float32)
    goi = np.asarray(genes_oi).astype(np.int64)
    r = np.asarray(cut_local_cellxgene_ix).astype(np.int64)
    ix2 = np.asarray(cut_localcellxgene_ix).astype(np.int64)
    j = np.asarray(cut_local_gene_ix).astype(np.int64)
    hsw = np.asarray(height_slope_w, np.float32)
    osw = np.asarray(overall_slope_w, np.float32)
    obase = np.asarray(overall_baseline, np.float32)
    sbase = np.asarray(spline_baseline, np.float32)
    n_cuts = x.shape[0]

    # spline bin / frac exactly as the reference computes them (f32)
    xs = np.clip(x, np.float32(0.0), np.float32(1.0 - 1e-6)) * np.float32(K)
    b = np.clip(np.floor(xs).astype(np.int32), 0, K - 1)
    m = b // 2                                              # double-bin [0,64)
    t = xs - (2 * m).astype(np.float32)                     # [0, 2)
    t7 = np.minimum(np.floor(t * np.float32(64.0)), 127).astype(np.int64)

    gene_r = (r % N_GOI).astype(np.int64)
    cell_r = r // N_GOI
    core = gene_r % N_CORES
    gl = gene_r // N_CORES                                  # [0, 63)
    r_loc = (cell_r * GPC + gl).astype(np.int32)            # [0, 63000)

    # bucket grid shared by all cores: 128 buckets (half, b//2) per core
    NB = 2 * M2
    half = (r_loc >= HALF).astype(np.int64)
    key = core * NB + half * M2 + m
    cnt = np.bincount(key, minlength=N_CORES * NB).reshape(N_CORES, NB)
    slots_b = (cnt.max(axis=0) + SLOT - 1) // SLOT          # [128]
    slots_b = np.maximum(slots_b, 1)
    h0 = int(slots_b[:M2].sum())
    h0r = ((h0 + GS - 1) // GS) * GS
    h1 = int(slots_b[M2:].sum())
    h1r = ((h1 + GS - 1) // GS) * GS
    off_b = np.zeros(NB + 1, np.int64)
    off_b[1:M2 + 1] = np.cumsum(slots_b[:M2])
    off_b[M2 + 1:] = h0r + np.cumsum(slots_b[M2:])
    # bucket slot ranges; extend last bucket of each half over region padding
    starts = off_b[:NB].copy()
    starts[M2] = h0r
    ends = off_b[1:].copy()
    ends[M2 - 1] = h0r
    ends[NB - 1] = h0r + h1r
    T_pad = h0r + h1r
    G = T_pad // GS
    half_of_group = [0 if g * GS < h0r else 1 for g in range(G)]

    order = np.argsort(key, kind="stable")
    key_s = key[order]
    bucket_start = np.searchsorted(key_s, np.arange(N_CORES * NB))
    rank = np.arange(n_cuts) - bucket_start[key_s]
    bloc = key_s % NB
    slot = starts[bloc] + rank // SLOT
    part = rank % SLOT
    core_s = key_s // NB

    flat = core_s * (SLOT * T_pad) + part * T_pad + slot
    g1o = np.full(N_CORES * SLOT * T_pad, HALF, np.int16)   # pad -> ones row
    g2o = np.full(N_CORES * SLOT * T_pad, N_GOI, np.uint16)
    g1o[flat] = (r_loc[order] - (bloc >= M2) * HALF).astype(np.int16)
    g2o[flat] = (j[order] | (t7[order] << 9)).astype(np.uint16)
    g1o = g1o.reshape(N_CORES, SLOT, T_pad)
    g2o = g2o.view(np.int16).reshape(N_CORES, SLOT, T_pad)
    n_pad = N_CORES * SLOT * T_pad - n_cuts

    # compact int16 index streams for dma_gather: element e (= slot*128 + part
    # within a group) at [e%16, e//16]; the 8x partition-block replication the
    # engine wants is done on device.
    def wrap_idx(a):  # a: [SLOT, T_pad] (partition, slot)
        e = np.ascontiguousarray(a.T).reshape(G, GC)          # [G, 8192]
        w = e.reshape(G, GC // 16, 16).transpose(0, 2, 1)     # [G, 16, 512]
        return np.ascontiguousarray(w.transpose(1, 0, 2).reshape(16, G * IW))

    # per-gene params
    W_oi = hsw[goi]                                          # [500, 10, 129]
    ec = np.ones((N_GOI + 1, NK), np.float32)
    ec[:N_GOI] = np.exp(sbase[goi])
    ec = ec.astype(BF16)
    oswT = np.concatenate([osw.T, obase[None, :]], axis=0).astype(BF16)
    latw = np.concatenate(
        [latent.T, np.ones((1, N_CELLS), np.float32)], axis=0).astype(BF16)

    counts = np.bincount(ix2, minlength=N_CELLS * N_GT).reshape(N_CELLS, N_GT)
    # clip counts at 3 (2-bit packing); correct the rare overflow on host,
    # using the same bf16-rounded params the device sees.
    over_c, over_g = np.nonzero(counts > 3)
    corr = 0.0
    if len(over_c):
        lw = latw.astype(np.float32)                         # [11, 1000]
        ow = oswT.astype(np.float32)                         # [11, 5000]
        sc = lw[:, over_c].T @ ow                            # [n_over, 5000]
        mx = sc.max(axis=1)
        lse = np.log(np.exp(sc - mx[:, None]).sum(axis=1)) + mx
        srow = sc[np.arange(len(over_c)), over_g]
        corr = float(((counts[over_c, over_g] - 3.0) * (srow - lse)).sum())
    cts = np.minimum(counts, 3).astype(np.uint8)

    in_maps = []
    for kcore in range(N_CORES):
        sel = np.arange(kcore, N_GOI, N_CORES)
        wk = np.zeros((GPC, NL, NK), np.float32)
        wk[:len(sel)] = W_oi[sel]
        woiT = np.ascontiguousarray(
            wk.transpose(1, 0, 2).reshape(NL, GPC * NK)).astype(BF16)
        cs = cts[kcore * CPC:(kcore + 1) * CPC]
        Q = N_GT // 4
        cpk = (cs[:, :Q] | (cs[:, Q:2 * Q] << 2) | (cs[:, 2 * Q:3 * Q] << 4)
               | (cs[:, 3 * Q:] << 6)).astype(np.uint8)
        in_maps.append({
            "latw": latw,
            "lown": np.ascontiguousarray(latw[:, kcore * CPC:(kcore + 1) * CPC]),
            "woiT": woiT,
            "oswT": oswT,
            "ectab": ec,
            "cpk": cpk,
            "g1w": wrap_idx(g1o[kcore]),
            "g2w": wrap_idx(g2o[kcore]),
        })
    grid = (tuple(int(s) for s in starts), tuple(int(e) for e in ends),
            int(G), int(T_pad), tuple(half_of_group))
    return in_maps, grid, n_cuts, n_pad, corr


def _build_program(starts, ends, G, T_pad, half_of_group,
                   phases="ABC", b_variant="full", gq=0):
    import concourse.bacc as bacc
    import concourse.bass as bass
    import concourse.mybir as mybir
    import concourse.tile as tile

    f32 = mybir.dt.float32
    bf16 = mybir.dt.bfloat16
    i16 = mybir.dt.int16
    u8 = mybir.dt.uint8
    Alu = mybir.AluOpType
    Act = mybir.ActivationFunctionType
    Ax = mybir.AxisListType
    NB = 2 * M2
    WW = GPC * NK                    # 8127 table cols per cell

    nc = bacc.Bacc(None, target_bir_lowering=False)

    latw = nc.dram_tensor("latw", [NL + 1, N_CELLS], bf16, kind="ExternalInput")
    lown = nc.dram_tensor("lown", [NL + 1, CPC], bf16, kind="ExternalInput")
    woiT = nc.dram_tensor("woiT", [NL, WW], bf16, kind="ExternalInput")
    oswT = nc.dram_tensor("oswT", [NL + 1, N_GT], bf16, kind="ExternalInput")
    ec_d = nc.dram_tensor("ectab", [N_GOI + 1, NK], bf16, kind="ExternalInput")
    cpk_d = nc.dram_tensor("cpk", [CPC, N_GT // 4], u8, kind="ExternalInput")
    g1w_d = nc.dram_tensor("g1w", [16, G * IW], i16, kind="ExternalInput")
    g2w_d = nc.dram_tensor("g2w", [16, G * IW], i16, kind="ExternalInput")
    out_d = nc.dram_tensor("out", [2, 1], f32, kind="ExternalOutput")

    with tile.TileContext(nc) as tc:
        with (
            tc.tile_pool(name="dram", bufs=1, space="DRAM") as dpool,
            tc.tile_pool(name="outer", bufs=1) as lpool,
            tc.tile_pool(name="psum", bufs=4, space="PSUM") as ppool,
        ):
            A_tab = dpool.tile([2 * HALFP, ES], bf16)
            ECT = dpool.tile([N_GOI + 1, ES], bf16)

            latw_sb = lpool.tile([NL + 1, N_CELLS], bf16)
            nc.sync.dma_start(latw_sb[:], latw[:])
            lown_sb = lpool.tile([NL + 1, CPC], bf16)
            nc.sync.dma_start(lown_sb[:], lown[:])
            accI = lpool.tile([SLOT, G], f32)
            nc.vector.memset(accI[:], 0.0)
            accS = lpool.tile([SLOT, G], f32)
            nc.vector.memset(accS[:], 0.0)
            ovacc = lpool.tile([SLOT, 4], f32)
            nc.vector.memset(ovacc[:], 0.0)
            i1_all = lpool.tile([SLOT, G * IW], i16)
            i2_all = lpool.tile([SLOT, G * IW], i16)
            for b8 in range(8):
                nc.sync.dma_start(i1_all[16 * b8:16 * (b8 + 1), :], g1w_d[:])
                nc.sync.dma_start(i2_all[16 * b8:16 * (b8 + 1), :], g2w_d[:])
            # extract the 7-bit t codes from block 0 (mask after the shift:
            # the ALU may promote int16 with sign extension), then permute the
            # wrapped e-order stream into slot-order [SLOT, T_pad]
            t_wr = lpool.tile([16, G * IW], i16)
            nc.vector.tensor_scalar(out=t_wr[:], in0=i2_all[0:16, :],
                                    scalar1=9, scalar2=127,
                                    op0=Alu.logical_shift_right,
                                    op1=Alu.bitwise_and)
            t_slot = lpool.tile([SLOT, T_pad], i16)
            src_all = t_wr[:].rearrange("i (g s q) -> i g s q", s=GS, q=8)
            for q in range(8):
                dst_v = t_slot[q * 16:(q + 1) * 16, :].rearrange(
                    "i (g s) -> i g s", s=GS)
                nc.sync.dma_start(dst_v, src_all[:, :, :, q])
            # strip the t bits so i2 becomes a clean gather index
            nc.vector.tensor_scalar(out=i2_all[:], in0=i2_all[:],
                                    scalar1=511, scalar2=None,
                                    op0=Alu.bitwise_and)

            # EC table: pad 129-elem rows into the 256-elem gather layout
            nc.sync.dma_start(ECT[:, 0:NK], ec_d[:])
            ones_sb = lpool.tile([2, ES], bf16)
            nc.vector.memset(ones_sb[:], 1.0)
            nc.sync.dma_start(A_tab[HALF:HALF + 1, :], ones_sb[0:1, :])
            nc.sync.dma_start(A_tab[2 * HALFP - 1:2 * HALFP, :], ones_sb[1:2, :])

            # ---------------- Phase A: build EA = exp(latent . w) ----------
            if "A" in phases:
              with tc.tile_pool(name="build", bufs=3) as bpool:
                woi_sb = bpool.tile([NL, WW], bf16, tag="woi")
                nc.sync.dma_start(woi_sb[:], woiT[:])
                for t in range(8):
                    stag = bpool.tile([CPC, WW], bf16, tag="stag")
                    sub = 0
                    while sub < WW:
                        sw = min(512, WW - sub)
                        ps = ppool.tile([CPC, 512], f32, tag="ps")
                        nc.tensor.matmul(
                            out=ps[:, :sw],
                            lhsT=latw_sb[0:NL, t * CPC:(t + 1) * CPC],
                            rhs=woi_sb[:, sub:sub + sw],
                            start=True, stop=True)
                        nc.scalar.activation(stag[:, sub:sub + sw], ps[:, :sw],
                                             Act.Exp)
                        sub += sw
                    r0 = t * CPC * GPC if t < 4 else HALFP + (t - 4) * CPC * GPC
                    dst = A_tab[r0:r0 + CPC * GPC, :].rearrange(
                        "(c g) e -> c g e", c=CPC)[:, :, 0:NK]
                    src = stag[:].rearrange("c (g e) -> c g e", e=NK)
                    nc.sync.dma_start(dst, src)

            # ---------------- Phase C: overall (softmax) term --------------
            if "C" in phases:
              with tc.tile_pool(name="ovp", bufs=1) as opool:
                osw_sb = opool.tile([NL + 1, N_GT], bf16)
                nc.sync.dma_start(osw_sb[:], oswT[:])
                scores = opool.tile([CPC, N_GT], f32)
                sub = 0
                while sub < N_GT:
                    sw = min(512, N_GT - sub)
                    ps = ppool.tile([CPC, 512], f32, tag="ps")
                    nc.tensor.matmul(
                        out=ps[:, :sw],
                        lhsT=lown_sb[:, :],
                        rhs=osw_sb[:, sub:sub + sw],
                        start=True, stop=True)
                    nc.vector.tensor_copy(scores[:, sub:sub + sw], ps[:, :sw])
                    sub += sw
                mrow = opool.tile([CPC, 1], f32)
                nc.vector.tensor_reduce(mrow[:], scores[:], axis=Ax.X, op=Alu.max)
                negm = opool.tile([CPC, 1], f32)
                nc.vector.tensor_scalar_mul(negm[:], mrow[:], -1.0)
                etrash = opool.tile([CPC, N_GT], bf16)
                sume = opool.tile([CPC, 1], f32)
                nc.scalar.activation(etrash[:], scores[:], Act.Exp,
                                     bias=negm[:], scale=1.0,
                                     accum_out=sume[:])
                lnse = opool.tile([CPC, 1], f32)
                nc.scalar.activation(lnse[:], sume[:], Act.Ln)
                lse = opool.tile([CPC, 1], f32)
                nc.vector.tensor_tensor(out=lse[:], in0=mrow[:], in1=lnse[:],
                                        op=Alu.add)
                Q = N_GT // 4
                cp_sb = opool.tile([CPC, Q], u8)
                nc.sync.dma_start(cp_sb[:], cpk_d[:])
                for q in range(4):
                    q8 = opool.tile([CPC, Q], u8, tag=f"q8_{q}")
                    nc.vector.tensor_scalar(out=q8[:], in0=cp_sb[:],
                                            scalar1=2 * q, scalar2=3,
                                            op0=Alu.logical_shift_right,
                                            op1=Alu.bitwise_and)
                    cq = opool.tile([CPC, Q], f32, tag=f"cq_{q}")
                    nc.vector.tensor_copy(cq[:], q8[:])
                    nc.vector.scalar_tensor_tensor(
                        out=scores[:, q * Q:(q + 1) * Q],
                        in0=scores[:, q * Q:(q + 1) * Q],
                        scalar=lse[:], in1=cq[:],
                        op0=Alu.subtract, op1=Alu.mult,
                        accum_out=ovacc[0:CPC, q:q + 1])

            # ---------------- Phase B: per-cut spline likelihood -----------
            with tc.tile_pool(name="main", bufs=2) as mpool:
                for g in range(G if "B" in phases else 0):
                    s0, s1 = g * GS, (g + 1) * GS
                    hf = half_of_group[g]
                    ha = mpool.tile([SLOT, GS, ES], bf16, tag="ha")
                    if b_variant == "none":
                        nc.vector.memset(ha[:], 0.5)
                    else:
                        nc.gpsimd.dma_gather(
                            out_ap=ha[:],
                            in_ap=A_tab[hf * HALFP:(hf + 1) * HALFP, :],
                            idxs_ap=i1_all[:, g * IW:(g + 1) * IW],
                            num_idxs=GC, num_idxs_reg=GC,
                            elem_size=ES, single_packet=False)
                    if b_variant == "g1":
                        nc.vector.tensor_reduce(accI[:, g:g + 1],
                                                ha[:, :, 0:NK],
                                                axis=Ax.XY, op=Alu.add)
                        continue
                    hc = mpool.tile([SLOT, GS, ES], bf16, tag="hc")
                    if b_variant == "none":
                        nc.vector.memset(hc[:], 0.5)
                    else:
                        nc.gpsimd.dma_gather(
                            out_ap=hc[:], in_ap=ECT[:],
                            idxs_ap=i2_all[:, g * IW:(g + 1) * IW],
                            num_idxs=GC, num_idxs_reg=GC,
                            elem_size=ES, single_packet=False,
                            queue_num=gq)
                    if b_variant == "g1g2":
                        nc.vector.tensor_reduce(accI[:, g:g + 1],
                                                ha[:, :, 0:NK],
                                                axis=Ax.XY, op=Alu.add)
                        continue
                    # u = EA * EC, in place (bf16)
                    nc.vector.tensor_tensor(
                        out=ha[:, :, 0:NK], in0=ha[:, :, 0:NK],
                        in1=hc[:, :, 0:NK], op=Alu.mult)

                    S0t = mpool.tile([SLOT, GS], f32, tag="S0")
                    nc.vector.tensor_reduce(S0t[:], ha[:, :, 0:NK],
                                            axis=Ax.X, op=Alu.add)
                    endst = mpool.tile([SLOT, GS], f32, tag="ends")
                    nc.vector.tensor_tensor(out=endst[:], in0=ha[:, :, 0],
                                            in1=ha[:, :, K], op=Alu.add)
                    Stt = mpool.tile([SLOT, GS], f32, tag="St")
                    nc.vector.scalar_tensor_tensor(
                        out=Stt[:], in0=endst[:], scalar=-0.5, in1=S0t[:],
                        op0=Alu.mult, op1=Alu.add)
                    if b_variant == "ub":
                        lgS = mpool.tile([SLOT, GS], f32, tag="lgS")
                        nc.scalar.activation(lgS[:], Stt[:], Act.Ln,
                                             accum_out=accS[:, g:g + 1])
                        continue

                    pr = mpool.tile([SLOT, GS, 3], f32, tag="pr")
                    for bb in range(NB):
                        lo = max(starts[bb], s0)
                        hi = min(ends[bb], s1)
                        if lo >= hi:
                            continue
                        col = 2 * (bb % M2)
                        nc.vector.tensor_copy(
                            pr[:, lo - s0:hi - s0, :],
                            ha[:, lo - s0:hi - s0, col:col + 3])

                    tf = mpool.tile([SLOT, GS], f32, tag="tf")
                    nc.vector.tensor_copy(tf[:], t_slot[:, s0:s1])
                    tq = mpool.tile([SLOT, GS], f32, tag="tq")
                    nc.vector.tensor_scalar(out=tq[:], in0=tf[:],
                                            scalar1=0.5, scalar2=1.0 / 64.0,
                                            op0=Alu.add, op1=Alu.mult)
                    av = mpool.tile([SLOT, GS], f32, tag="av")
                    nc.vector.tensor_scalar_min(av[:], tq[:], 1.0)
                    bv = mpool.tile([SLOT, GS], f32, tag="bv")
                    nc.vector.tensor_scalar(out=bv[:], in0=tq[:],
                                            scalar1=1.0, scalar2=0.0,
                                            op0=Alu.subtract, op1=Alu.max)
                    d10 = mpool.tile([SLOT, GS], f32, tag="d10")
                    nc.vector.tensor_tensor(out=d10[:], in0=pr[:, :, 1],
                                            in1=pr[:, :, 0], op=Alu.subtract)
                    d21 = mpool.tile([SLOT, GS], f32, tag="d21")
                    nc.vector.tensor_tensor(out=d21[:], in0=pr[:, :, 2],
                                            in1=pr[:, :, 1], op=Alu.subtract)
                    m1 = mpool.tile([SLOT, GS], f32, tag="m1")
                    nc.vector.tensor_tensor(out=m1[:], in0=av[:],
                                            in1=d10[:], op=Alu.mult)
                    m2 = mpool.tile([SLOT, GS], f32, tag="m2")
                    nc.vector.tensor_tensor(out=m2[:], in0=bv[:],
                                            in1=d21[:], op=Alu.mult)
                    It1 = mpool.tile([SLOT, GS], f32, tag="It1")
                    nc.vector.tensor_tensor(out=It1[:], in0=m1[:],
                                            in1=pr[:, :, 0], op=Alu.add)
                    It = mpool.tile([SLOT, GS], f32, tag="It")
                    nc.vector.tensor_tensor(out=It[:], in0=It1[:],
                                            in1=m2[:], op=Alu.add)
                    lgI = mpool.tile([SLOT, GS], f32, tag="lgI")
                    nc.scalar.activation(lgI[:], It[:], Act.Ln,
                                         accum_out=accI[:, g:g + 1])
                    lgS = mpool.tile([SLOT, GS], f32, tag="lgS")
                    nc.scalar.activation(lgS[:], Stt[:], Act.Ln,
                                         accum_out=accS[:, g:g + 1])

            # -------- final reduction to two scalars --------
            accd = lpool.tile([SLOT, 1], f32)
            nc.vector.tensor_reduce(accd[:], accI[:], axis=Ax.X, op=Alu.add)
            accs1 = lpool.tile([SLOT, 1], f32)
            nc.vector.tensor_reduce(accs1[:], accS[:], axis=Ax.X, op=Alu.add)
            acc1 = lpool.tile([SLOT, 1], f32)
            nc.vector.tensor_tensor(out=acc1[:], in0=accd[:], in1=accs1[:],
                                    op=Alu.subtract)
            ovs = lpool.tile([SLOT, 1], f32)
            nc.vector.tensor_reduce(ovs[:], ovacc[:], axis=Ax.X, op=Alu.add)
            comb = lpool.tile([SLOT, 2], f32)
            nc.vector.tensor_copy(comb[:, 0:1], acc1[:])
            nc.vector.tensor_copy(comb[:, 1:2], ovs[:])
            ones1 = lpool.tile([SLOT, 1], f32)
            nc.vector.memset(ones1[:], 1.0)
            pres = ppool.tile([2, 1], f32, tag="pres")
            nc.tensor.matmul(out=pres[:], lhsT=comb[:], rhs=ones1[:],
                             start=True, stop=True)
            res_sb = lpool.tile([2, 1], f32)
            nc.vector.tensor_copy(res_sb[:], pres[:])
            nc.sync.dma_start(out_d[:], res_sb[:])

    nc.finalize()
    return nc


def kernel(**inputs) -> np.ndarray:
    from concourse.bass_utils import run_bass_kernel_spmd

    in_maps, grid, n_cuts, n_pad, corr = _host_prep(**inputs)
    if grid in _PROGRAM_CACHE:
        nc = _PROGRAM_CACHE[grid]
    else:
        nc = _build_program(*grid)
        _PROGRAM_CACHE[grid] = nc

    res = run_bass_kernel_spmd(nc, in_maps, list(range(N_CORES)))
    total = corr
    for kcore in range(N_CORES):
        o = np.asarray(res.results[kcore]["out"], np.float64)
        total += o[0, 0] + o[1, 0]
    total += n_cuts * (np.log(128.0) + np.log(5000.0)) + n_pad * np.log(128.0)
    return np.float32(-total)
